# revision 39
# baseline (speedup 1.0000x reference)
"""Scatter-add (A.at[index].add(B)) on 8 trn2 NeuronCores.

Strategy: value-range windowing + snake-dealt sharding. Host sorts rows by
index value into 128-value windows, deals windows to the 8 cores in snake
order of row count (so every core sees a near-identical count profile, which
minimizes the SPMD shared padding), and runs all floating-point work on
device via one-hot selection matmuls. The host only permutes/pads/quantizes
inputs and concatenates the per-core output slices.

Transport is fp16 end-to-end (B rows, embedded A rows, output), which halves
HBM traffic versus fp32; worst-case output error is ~1e-3 relative, far
inside the 2e-2 gate.

Device program per window (window = 128 consecutive output rows; windows are
packed at SLOT granularity, so consecutive windows share boundary 128-row
chunks and the only padding is the per-position cross-core max):
  S[p, v, j] = (idx_rel[p, j] == v)     batched DVE is_equal; the selection
      is laid out v-major so every operand's innermost axis is packed, which
      enables the DVE 2x half-cycle mode (broadcast-last layouts run 1x)
  psum[v, d] = sum_j S[:, :, j]^T @ B_j  K PSUM-accumulated matmuls
  out[v, d]  = psum                      batched fp32->fp16 copy + store

A-handling: every window's 128 A rows are embedded in its B padding slots
with idx_rel = v (the window's slot range always fits count+128 rows), so
the selection matmul adds A for free and there is no separate A path.

DMAs are grouped ~6 windows per transfer (~1.2MB) because each DMA holds
the descriptor-generation stage (~630ns) exclusively; per-window DMAs would
bottleneck there. The final groups shrink to 1 window so the critical tail
(last-arriving data -> matmul -> copy -> store) is minimal.

The TRN2 instruction encodings carry a limited number of semaphore waits, so
the index table ships in one DRAM tensor loaded by a single DMA and the
module is built via Bacc (whose compile() legalizes multi-wait
instructions). The per-K iota compare tables are expanded on device by DVE
from a 128-column base (gpsimd iota crashes the exec unit on this silicon,
and shipping them by DMA costs stream time).
"""

import sys

import numpy as np

sys.path.insert(0, "/opt/trn_rl_repo")

N, M, D = 100000, 500000, 128
P = 128
NCORES = 8

W_GLOBAL = (N + P - 1) // P              # 782 value-windows
WPC = (W_GLOBAL + NCORES - 1) // NCORES  # 98 windows per core
W_PAD = WPC * NCORES                     # 784
N_PAD = W_PAD * P                        # 100352 output rows before trimming

# Windows per DMA load group: bulk groups of 7, then a fine-grained tail so
# the last-arriving transfer gates almost no compute.
BULK = 14
GROUPS = [6] * BULK + [4, 3, 2, 2, 1, 1, 1]
assert sum(GROUPS) == WPC
GSTART = np.concatenate([[0], np.cumsum(GROUPS)])
NG = len(GROUPS)

_BUILT = {}
_LAST_RES = None


def layout(need):
    """Slot-granular layout shared by host packing and device build.

    Window at position pos owns slots [S[pos], S[pos]+need[pos]); consecutive
    windows share boundary chunks (a shared chunk gets one masked matmul per
    window), so the only padding is the per-position cross-core max.
    """
    need = np.asarray(need, np.int64)
    S = np.concatenate([[0], np.cumsum(need)])
    c0 = S[:-1] // P                              # first chunk per position
    k_w = (S[:-1] + need + P - 1) // P - c0       # chunks spanned
    tb = np.concatenate([[0], np.cumsum(k_w)])    # table column starts
    tot_chunks = int((S[-1] + P - 1) // P)
    glo = c0[GSTART[:-1]]                         # group first chunk
    ghi = c0[GSTART[1:] - 1] + k_w[GSTART[1:] - 1]  # group end chunk (excl)
    return S, c0, k_w, tb, tot_chunks, glo, ghi


def build_bass(need, bufs_big=7, bufs_sel=7, bufs_small=5, bufs_psum=8):
    """Build the SPMD Bass module for the per-position slot needs."""
    from concourse import bacc, mybir, tile

    assert len(need) == WPC
    f32 = mybir.dt.float32
    f16 = mybir.dt.float16

    S, c0, k_w, tb, tot_chunks, glo, ghi = layout(need)
    # Distinct chunk spans ordered by first use, so each DVE-expanded iota
    # table is ready by the first is_equal that needs it.
    ks = list(dict.fromkeys(int(k) for k in k_w))
    ib = int(tb[-1])                              # iota base column
    cw = ib + P

    nc = bacc.Bacc("TRN2", target_bir_lowering=False, debug=False)

    b_d = nc.dram_tensor("b_pad", [P, tot_chunks * P], f16, kind="ExternalInput").ap()
    c_d = nc.dram_tensor("consts", [P, cw], f16, kind="ExternalInput").ap()
    out_d = nc.dram_tensor("out", [P, WPC * P], f16, kind="ExternalOutput").ap()

    sg_max = int((ghi - glo).max()) * P

    with tile.TileContext(nc) as tc:
        with (
            tc.tile_pool(name="const", bufs=1) as cpool,
            tc.tile_pool(name="big", bufs=bufs_big) as bpool,
            tc.tile_pool(name="sel", bufs=bufs_sel) as selpool,
            tc.tile_pool(name="small", bufs=bufs_small) as spool,
            tc.tile_pool(name="psum", bufs=bufs_psum, space="PSUM") as ppool,
        ):
            c_t = cpool.tile([P, cw], f16)
            nc.sync.dma_start(out=c_t[:], in_=c_d[:])
            # Per-k iota tables (iota_k[p, v*k+j] = v) expanded on DVE from a
            # 128-column DMA'd base instead of spending DMA bytes on each.
            io_t = {}
            for k in ks:
                io_t[k] = cpool.tile([P, P * k], f16, name=f"io{k}")
                nc.vector.tensor_copy(
                    out=io_t[k][:].rearrange("p (v j) -> p v j", j=k),
                    in_=c_t[:, ib : ib + P].to_broadcast([P, P, k]),
                )

            # Store groups: one o_t + store per bulk load group, but two
            # merged stores for the whole tail, issued from the sync queue
            # which is idle by then (per-store SEQ overhead, ~700ns of
            # descriptor generation each, otherwise serializes the tail).
            # PSUM batches: 4 windows share one 2KB PSUM bank so one fp32->
            # fp16 copy covers 4 windows (the copy cost is dominated by a
            # ~185ns SBUF access latency, so batching quarters it).
            t0 = int(GSTART[BULK])                  # first tail position
            sbatch = []                             # (start, end) psum batches
            for g in range(BULK):
                p0, p1 = int(GSTART[g]), int(GSTART[g + 1])
                sbatch += [(p0, min(p0 + 4, p1)), (min(p0 + 4, p1), p1)]
            sbatch += [(p, min(p + 4, WPC)) for p in range(t0, WPC, 4)]
            batch_of = {}
            for i, (s0, s1) in enumerate(sbatch):
                for pos in range(s0, s1):
                    batch_of[pos] = i

            tmid = t0 + 8                           # tail split points
            tl = t0 + 12
            stores = []                             # deferred bulk stores
            ob_bounds = [0, 5, 10, BULK]            # merged-store extents
            OALLOC = {ob_bounds[i]: ob_bounds[i + 1]
                      for i in range(len(ob_bounds) - 1)}
            b_t = o_t = o_t2 = ps4 = None
            b_prev = glo_prev = None
            ghi_prev = 0
            for g in range(NG):
                p0, p1 = int(GSTART[g]), int(GSTART[g + 1])
                g0 = int(glo[g])
                sg = (int(ghi[g]) - g0) * P
                # A chunk shared with the previous group is already on chip;
                # skip re-loading it and read it from the previous tile.
                skip = (ghi_prev - g0) * P if g > 0 and ghi_prev > g0 else 0
                b_t = bpool.tile([P, sg_max], f16, tag="b")
                nc.sync.dma_start(
                    out=b_t[:, skip:sg],
                    in_=b_d[:, g0 * P + skip : g0 * P + sg],
                )
                if g < BULK:
                    # The deferred-store drain is paced by the ~625ns/DMA
                    # descriptor stage, so merge several groups per store.
                    if g in OALLOC:
                        ow = int(GSTART[OALLOC[g]]) - p0
                        o_t = spool.tile([P, ow * P], f16, tag="o")
                        ob = p0
                elif g == BULK:
                    o_t = spool.tile([P, (WPC - t0) * P], f16, tag="o")
                    ob = t0                         # o_t covers the tail

                # One batched is_equal per run of equal chunk count
                # (per-window DVE ops pay decode + semaphore overhead each,
                # which throttles the pipeline's issue rate).
                runs = []
                for pos in range(p0, p1):
                    if (runs and k_w[pos] == runs[-1][2]
                            and runs[-1][1] - runs[-1][0] < 4):
                        runs[-1][1] = pos + 1
                    else:
                        runs.append([pos, pos + 1, int(k_w[pos])])
                for r0, r1, k in runs:
                    nw = r1 - r0
                    t_c = int(tb[r0])
                    s_t = selpool.tile([P, nw, P, k], f16, tag="s")
                    nc.vector.tensor_tensor(
                        out=s_t[:],
                        in0=c_t[:, t_c : t_c + nw * k]
                        .rearrange("p (u j) -> p u j", j=k)
                        .to_broadcast([P, nw, k, P])
                        .transpose([0, 1, 3, 2]),
                        in1=io_t[k][:]
                        .rearrange("p (v j) -> p v j", j=k)
                        .unsqueeze(1)
                        .broadcast_to([P, nw, P, k]),
                        op=mybir.AluOpType.is_equal,
                    )
                    for u, pos in enumerate(range(r0, r1)):
                        s0, s1 = sbatch[batch_of[pos]]
                        if pos == s0:
                            ps4 = ppool.tile([P, (s1 - s0) * P], f32, tag="ps")
                        po = (pos - s0) * P
                        for j in range(k):
                            c = int(c0[pos]) + j
                            if c * P < g0 * P + skip and b_prev is not None:
                                rhs = b_prev[
                                    :,
                                    (c - glo_prev) * P : (c - glo_prev + 1) * P,
                                ]
                            else:
                                rhs = b_t[:, (c - g0) * P : (c - g0 + 1) * P]
                            nc.tensor.matmul(
                                out=ps4[:, po : po + P],
                                lhsT=s_t[:, u, :, j],
                                rhs=rhs,
                                start=(j == 0),
                                stop=(j == k - 1),
                            )
                        if pos == s1 - 1:
                            dst, oo = o_t, (s0 - ob) * P
                            if g >= BULK and s0 >= tl:
                                # tail copies past the split go on DVE (idle
                                # by then) so the Act copy backlog doesn't
                                # delay PSUM recycling and the final store
                                nc.vector.tensor_copy(
                                    out=dst[:, oo : oo + (s1 - s0) * P],
                                    in_=ps4[:],
                                )
                            else:
                                nc.scalar.copy(
                                    out=dst[:, oo : oo + (s1 - s0) * P],
                                    in_=ps4[:],
                                )
                if g + 1 in OALLOC or g == BULK - 1:
                    stores.append((ob, p1, o_t))
                b_prev, glo_prev, ghi_prev = b_t, g0, int(ghi[g])
            # All stores are deferred to the end of the program: the output
            # drain then runs AFTER the b stream on the exclusive DMA device,
            # so the tail windows' compute hides under the drain instead of
            # extending the makespan. Alternate queues so per-store
            # descriptor-generation (~680ns) pipelines against transfers.
            # The tail store ends the program: issue it first among the
            # deferred stores so it doesn't queue behind their descriptor
            # generation; the bulk stores fill the stream around it.
            nc.sync.dma_start(out=out_d[:, t0 * P :], in_=o_t[:])
            for i, (p0, p1, ot) in enumerate(stores):
                q = nc.scalar if i % 2 else nc.sync
                q.dma_start(out=out_d[:, p0 * P : p1 * P], in_=ot[:])
    nc.compile()
    return nc


def shard_inputs(index, A, B):
    """Sort rows by index value into windows, snake-deal windows to cores by
    row count, pick per-position chunk counts, embed A rows in the padding."""
    idx = np.asarray(index).astype(np.int64).ravel()
    A = np.asarray(A, dtype=np.float32)
    B = np.ascontiguousarray(np.asarray(B, dtype=np.float32))

    order = np.argsort(idx, kind="stable")
    sidx = idx[order]
    bounds = np.searchsorted(sidx, np.arange(0, N_PAD + 1, P)).astype(np.int64)
    counts = np.diff(bounds)                      # (W_PAD,) rows per window

    # Snake-deal windows (heaviest first) across cores: window with global
    # count-rank r goes to core snake(r % 8) at position r // 8.
    rank = np.argsort(-counts, kind="stable")     # rank -> window id
    core_of_rank = np.tile(
        np.concatenate([np.arange(NCORES), np.arange(NCORES)[::-1]]),
        (W_PAD + 2 * NCORES - 1) // (2 * NCORES),
    )[:W_PAD]
    pos_of_rank = np.arange(W_PAD) // NCORES
    wcore = np.empty(W_PAD, np.int64)             # window id -> core
    wpos = np.empty(W_PAD, np.int64)              # window id -> position
    wcore[rank] = core_of_rank
    wpos[rank] = pos_of_rank
    wid = np.empty((NCORES, WPC), np.int64)       # (core, pos) -> window id
    wid[wcore, wpos] = np.arange(W_PAD)
    pos_counts = counts[wid]                      # (core, pos)

    # Slots per position, shared across cores: count + 128 A rows must fit.
    need = pos_counts.max(axis=0) + P
    S, c0, k_w, tb, tot_chunks, glo, ghi = layout(need)

    win = (sidx // P).astype(np.int64)
    qpos = np.arange(M, dtype=np.int64) - bounds[win]
    core = wcore[win]
    pos = wpos[win]
    slot = S[pos] + qpos                          # global slot of each B row
    p = slot % P
    ch = slot // P                                # global chunk
    j = ch - c0[pos]                              # window-local chunk

    # b layout: (core, p, chunk, d).
    b_all = np.zeros((NCORES, P, tot_chunks, P), np.float16)
    b_all[core, p, ch] = B[order].astype(np.float16)

    # consts layout: [idx table (p, window-local chunk) | 128-col iota base]
    ib = int(tb[-1])
    cw = ib + P
    consts_arr = np.full((NCORES, P, cw), -1.0, np.float16)
    consts_arr[:, :, ib : ib + P] = np.arange(P, dtype=np.float16)
    consts_arr[core, p, tb[pos] + j] = (sidx - win * P).astype(np.float16)

    a_pad = np.zeros((N_PAD, D), np.float32)
    a_pad[:N] = A

    # Embed each window's 128 A rows right after its B rows.
    ce = np.repeat(np.arange(NCORES), WPC * P)
    pe_ = np.tile(np.repeat(np.arange(WPC), P), NCORES)
    v3 = np.tile(np.arange(P), NCORES * WPC)
    w3 = wid[ce, pe_]
    s3 = S[pe_] + pos_counts[ce, pe_] + v3
    a_rows = a_pad[w3 * P + v3].astype(np.float16)
    b_all[ce, s3 % P, s3 // P] = a_rows
    consts_arr[ce, s3 % P, tb[pe_] + s3 // P - c0[pe_]] = v3.astype(np.float16)

    b_flat = b_all.reshape(NCORES, P, tot_chunks * P)
    in_maps = [
        {"b_pad": b_flat[c], "consts": consts_arr[c]} for c in range(NCORES)
    ]
    return tuple(int(n) for n in need), wid, in_maps


def assemble_out(results, wid):
    """results[c]["out"] is (v, pos*128+d); route each position's window back
    to its window id's rows."""
    full = np.empty((N_PAD, D), np.float32)
    rows = full.reshape(W_PAD, P, D)
    for c in range(NCORES):
        o = np.asarray(results[c]["out"], dtype=np.float32)
        o = o.reshape(P, WPC, D).transpose(1, 0, 2)   # (pos, v, d)
        rows[wid[c]] = o
    return full[:N]


def kernel(index, A, B):
    from concourse.bass_utils import run_bass_kernel_spmd

    key, wid, in_maps = shard_inputs(index, A, B)
    if key not in _BUILT:
        _BUILT[key] = build_bass(key)
    nc = _BUILT[key]

    res = run_bass_kernel_spmd(nc, in_maps, list(range(NCORES)))
    global _LAST_RES
    _LAST_RES = res
    full = assemble_out(res.results, wid)
    return np.ascontiguousarray(full.astype(np.float32))


# revision 40
# speedup vs baseline: 1.0212x; 1.0212x over previous
"""Scatter-add (A.at[index].add(B)) on 8 trn2 NeuronCores.

Strategy: value-range windowing + snake-dealt sharding. Host sorts rows by
index value into 128-value windows, deals windows to the 8 cores in snake
order of row count (so every core sees a near-identical count profile, which
minimizes the SPMD shared padding), and runs all floating-point work on
device via one-hot selection matmuls. The host only permutes/pads/quantizes
inputs and concatenates the per-core output slices.

Transport is fp16 end-to-end (B rows, embedded A rows, output), which halves
HBM traffic versus fp32; worst-case output error is ~1e-3 relative, far
inside the 2e-2 gate.

Device program per window (window = 128 consecutive output rows; windows are
packed at SLOT granularity, so consecutive windows share boundary 128-row
chunks and the only padding is the per-position cross-core max):
  S[p, v, j] = (idx_rel[p, j] == v)     batched DVE is_equal; the selection
      is laid out v-major so every operand's innermost axis is packed, which
      enables the DVE 2x half-cycle mode (broadcast-last layouts run 1x)
  psum[v, d] = sum_j S[:, :, j]^T @ B_j  K PSUM-accumulated matmuls
  out[v, d]  = psum                      batched fp32->fp16 copy + store

A-handling: every window's 128 A rows are embedded in its B padding slots
with idx_rel = v (the window's slot range always fits count+128 rows), so
the selection matmul adds A for free and there is no separate A path.

DMAs are grouped ~6 windows per transfer (~1.2MB) because each DMA holds
the descriptor-generation stage (~630ns) exclusively; per-window DMAs would
bottleneck there. The final groups shrink to 1 window so the critical tail
(last-arriving data -> matmul -> copy -> store) is minimal.

The TRN2 instruction encodings carry a limited number of semaphore waits, so
the index table ships in one DRAM tensor loaded by a single DMA and the
module is built via Bacc (whose compile() legalizes multi-wait
instructions). The per-K iota compare tables are expanded on device by DVE
from a 128-column base (gpsimd iota crashes the exec unit on this silicon,
and shipping them by DMA costs stream time).
"""

import sys

import numpy as np

sys.path.insert(0, "/opt/trn_rl_repo")

N, M, D = 100000, 500000, 128
P = 128
NCORES = 8

W_GLOBAL = (N + P - 1) // P              # 782 value-windows
WPC = (W_GLOBAL + NCORES - 1) // NCORES  # 98 windows per core
W_PAD = WPC * NCORES                     # 784
N_PAD = W_PAD * P                        # 100352 output rows before trimming

# Windows per DMA load group: bulk groups of 7, then a fine-grained tail so
# the last-arriving transfer gates almost no compute.
BULK = 14
GROUPS = [6] * BULK + [4, 3, 2, 2, 1, 1, 1]
assert sum(GROUPS) == WPC
GSTART = np.concatenate([[0], np.cumsum(GROUPS)])
NG = len(GROUPS)

_BUILT = {}
_LAST_RES = None


def layout(need):
    """Slot-granular layout shared by host packing and device build.

    Window at position pos owns slots [S[pos], S[pos]+need[pos]); consecutive
    windows share boundary chunks (a shared chunk gets one masked matmul per
    window), so the only padding is the per-position cross-core max.
    """
    need = np.asarray(need, np.int64)
    S = np.concatenate([[0], np.cumsum(need)])
    c0 = S[:-1] // P                              # first chunk per position
    k_w = (S[:-1] + need + P - 1) // P - c0       # chunks spanned
    tb = np.concatenate([[0], np.cumsum(k_w)])    # table column starts
    tot_chunks = int((S[-1] + P - 1) // P)
    glo = c0[GSTART[:-1]]                         # group first chunk
    ghi = c0[GSTART[1:] - 1] + k_w[GSTART[1:] - 1]  # group end chunk (excl)
    return S, c0, k_w, tb, tot_chunks, glo, ghi


def build_bass(need, bufs_big=7, bufs_sel=7, bufs_small=12, bufs_psum=8):
    """Build the SPMD Bass module for the per-position slot needs."""
    from concourse import bacc, mybir, tile

    assert len(need) == WPC
    f32 = mybir.dt.float32
    f16 = mybir.dt.float16

    S, c0, k_w, tb, tot_chunks, glo, ghi = layout(need)
    # Distinct chunk spans ordered by first use, so each DVE-expanded iota
    # table is ready by the first is_equal that needs it.
    ks = list(dict.fromkeys(int(k) for k in k_w))
    ib = int(tb[-1])                              # iota base column
    cw = ib + P

    nc = bacc.Bacc("TRN2", target_bir_lowering=False, debug=False)

    b_d = nc.dram_tensor("b_pad", [P, tot_chunks * P], f16, kind="ExternalInput").ap()
    c_d = nc.dram_tensor("consts", [P, cw], f16, kind="ExternalInput").ap()
    out_d = nc.dram_tensor("out", [P, WPC * P], f16, kind="ExternalOutput").ap()

    sg_max = int((ghi - glo).max()) * P

    with tile.TileContext(nc) as tc:
        with (
            tc.tile_pool(name="const", bufs=1) as cpool,
            tc.tile_pool(name="big", bufs=bufs_big) as bpool,
            tc.tile_pool(name="sel", bufs=bufs_sel) as selpool,
            tc.tile_pool(name="small", bufs=bufs_small) as spool,
            tc.tile_pool(name="psum", bufs=bufs_psum, space="PSUM") as ppool,
        ):
            c_t = cpool.tile([P, cw], f16)
            nc.sync.dma_start(out=c_t[:], in_=c_d[:])
            # Per-k iota tables (iota_k[p, v*k+j] = v) expanded on DVE from a
            # 128-column DMA'd base instead of spending DMA bytes on each.
            io_t = {}
            for k in ks:
                io_t[k] = cpool.tile([P, P * k], f16, name=f"io{k}")
                nc.vector.tensor_copy(
                    out=io_t[k][:].rearrange("p (v j) -> p v j", j=k),
                    in_=c_t[:, ib : ib + P].to_broadcast([P, P, k]),
                )

            # Store groups: one o_t + store per bulk load group, but two
            # merged stores for the whole tail, issued from the sync queue
            # which is idle by then (per-store SEQ overhead, ~700ns of
            # descriptor generation each, otherwise serializes the tail).
            # PSUM batches: 4 windows share one 2KB PSUM bank so one fp32->
            # fp16 copy covers 4 windows (the copy cost is dominated by a
            # ~185ns SBUF access latency, so batching quarters it).
            t0 = int(GSTART[BULK])                  # first tail position
            sbatch = []                             # (start, end) psum batches
            for g in range(BULK):
                p0, p1 = int(GSTART[g]), int(GSTART[g + 1])
                sbatch += [(p0, min(p0 + 4, p1)), (min(p0 + 4, p1), p1)]
            sbatch += [(p, min(p + 4, WPC)) for p in range(t0, WPC, 4)]
            batch_of = {}
            for i, (s0, s1) in enumerate(sbatch):
                for pos in range(s0, s1):
                    batch_of[pos] = i

            tmid = t0 + 8                           # tail split points
            tl = t0 + 12
            stores = []                             # deferred bulk stores
            ob_bounds = [0, 4, 8, 11, BULK]         # merged-store extents
            OALLOC = {ob_bounds[i]: ob_bounds[i + 1]
                      for i in range(len(ob_bounds) - 1)}
            b_t = o_t = o_t2 = ps4 = None
            b_prev = glo_prev = None
            ghi_prev = 0
            for g in range(NG):
                p0, p1 = int(GSTART[g]), int(GSTART[g + 1])
                g0 = int(glo[g])
                sg = (int(ghi[g]) - g0) * P
                # A chunk shared with the previous group is already on chip;
                # skip re-loading it and read it from the previous tile.
                skip = (ghi_prev - g0) * P if g > 0 and ghi_prev > g0 else 0
                b_t = bpool.tile([P, sg_max], f16, tag="b")
                nc.sync.dma_start(
                    out=b_t[:, skip:sg],
                    in_=b_d[:, g0 * P + skip : g0 * P + sg],
                )
                if g < BULK:
                    # The deferred-store drain is paced by the ~625ns/DMA
                    # descriptor stage, so merge several groups per store.
                    if g in OALLOC:
                        ow = int(GSTART[OALLOC[g]]) - p0
                        o_t = spool.tile([P, ow * P], f16, tag="o")
                        ob = p0
                elif g == BULK:
                    o_t = spool.tile([P, (WPC - t0) * P], f16, tag="o")
                    ob = t0                         # o_t covers the tail

                # One batched is_equal per run of equal chunk count
                # (per-window DVE ops pay decode + semaphore overhead each,
                # which throttles the pipeline's issue rate).
                runs = []
                for pos in range(p0, p1):
                    if (runs and k_w[pos] == runs[-1][2]
                            and runs[-1][1] - runs[-1][0] < 4):
                        runs[-1][1] = pos + 1
                    else:
                        runs.append([pos, pos + 1, int(k_w[pos])])
                for r0, r1, k in runs:
                    nw = r1 - r0
                    t_c = int(tb[r0])
                    s_t = selpool.tile([P, nw, P, k], f16, tag="s")
                    nc.vector.tensor_tensor(
                        out=s_t[:],
                        in0=c_t[:, t_c : t_c + nw * k]
                        .rearrange("p (u j) -> p u j", j=k)
                        .to_broadcast([P, nw, k, P])
                        .transpose([0, 1, 3, 2]),
                        in1=io_t[k][:]
                        .rearrange("p (v j) -> p v j", j=k)
                        .unsqueeze(1)
                        .broadcast_to([P, nw, P, k]),
                        op=mybir.AluOpType.is_equal,
                    )
                    for u, pos in enumerate(range(r0, r1)):
                        s0, s1 = sbatch[batch_of[pos]]
                        if pos == s0:
                            ps4 = ppool.tile([P, (s1 - s0) * P], f32, tag="ps")
                        po = (pos - s0) * P
                        for j in range(k):
                            c = int(c0[pos]) + j
                            if c * P < g0 * P + skip and b_prev is not None:
                                rhs = b_prev[
                                    :,
                                    (c - glo_prev) * P : (c - glo_prev + 1) * P,
                                ]
                            else:
                                rhs = b_t[:, (c - g0) * P : (c - g0 + 1) * P]
                            nc.tensor.matmul(
                                out=ps4[:, po : po + P],
                                lhsT=s_t[:, u, :, j],
                                rhs=rhs,
                                start=(j == 0),
                                stop=(j == k - 1),
                            )
                        if pos == s1 - 1:
                            dst, oo = o_t, (s0 - ob) * P
                            if g >= BULK and s0 >= tl:
                                # tail copies past the split go on DVE (idle
                                # by then) so the Act copy backlog doesn't
                                # delay PSUM recycling and the final store
                                nc.vector.tensor_copy(
                                    out=dst[:, oo : oo + (s1 - s0) * P],
                                    in_=ps4[:],
                                )
                            else:
                                nc.scalar.copy(
                                    out=dst[:, oo : oo + (s1 - s0) * P],
                                    in_=ps4[:],
                                )
                if g + 1 in OALLOC or g == BULK - 1:
                    stores.append((ob, p1, o_t))
                b_prev, glo_prev, ghi_prev = b_t, g0, int(ghi[g])
            # All stores are deferred to the end of the program: the output
            # drain then runs AFTER the b stream on the exclusive DMA device,
            # so the tail windows' compute hides under the drain instead of
            # extending the makespan. Alternate queues so per-store
            # descriptor-generation (~680ns) pipelines against transfers.
            # The tail store ends the program: issue it first among the
            # deferred stores so it doesn't queue behind their descriptor
            # generation; the bulk stores fill the stream around it.
            nc.sync.dma_start(out=out_d[:, t0 * P :], in_=o_t[:])
            for i, (p0, p1, ot) in enumerate(stores):
                q = nc.scalar if i % 2 else nc.sync
                q.dma_start(out=out_d[:, p0 * P : p1 * P], in_=ot[:])
    nc.compile()
    return nc


def shard_inputs(index, A, B):
    """Sort rows by index value into windows, snake-deal windows to cores by
    row count, pick per-position chunk counts, embed A rows in the padding."""
    idx = np.asarray(index).astype(np.int64).ravel()
    A = np.asarray(A, dtype=np.float32)
    B = np.ascontiguousarray(np.asarray(B, dtype=np.float32))

    order = np.argsort(idx, kind="stable")
    sidx = idx[order]
    bounds = np.searchsorted(sidx, np.arange(0, N_PAD + 1, P)).astype(np.int64)
    counts = np.diff(bounds)                      # (W_PAD,) rows per window

    # Snake-deal windows (heaviest first) across cores: window with global
    # count-rank r goes to core snake(r % 8) at position r // 8.
    rank = np.argsort(-counts, kind="stable")     # rank -> window id
    core_of_rank = np.tile(
        np.concatenate([np.arange(NCORES), np.arange(NCORES)[::-1]]),
        (W_PAD + 2 * NCORES - 1) // (2 * NCORES),
    )[:W_PAD]
    pos_of_rank = np.arange(W_PAD) // NCORES
    wcore = np.empty(W_PAD, np.int64)             # window id -> core
    wpos = np.empty(W_PAD, np.int64)              # window id -> position
    wcore[rank] = core_of_rank
    wpos[rank] = pos_of_rank
    wid = np.empty((NCORES, WPC), np.int64)       # (core, pos) -> window id
    wid[wcore, wpos] = np.arange(W_PAD)
    pos_counts = counts[wid]                      # (core, pos)

    # Slots per position, shared across cores: count + 128 A rows must fit.
    need = pos_counts.max(axis=0) + P
    S, c0, k_w, tb, tot_chunks, glo, ghi = layout(need)

    win = (sidx // P).astype(np.int64)
    qpos = np.arange(M, dtype=np.int64) - bounds[win]
    core = wcore[win]
    pos = wpos[win]
    slot = S[pos] + qpos                          # global slot of each B row
    p = slot % P
    ch = slot // P                                # global chunk
    j = ch - c0[pos]                              # window-local chunk

    # b layout: (core, p, chunk, d).
    b_all = np.zeros((NCORES, P, tot_chunks, P), np.float16)
    b_all[core, p, ch] = B[order].astype(np.float16)

    # consts layout: [idx table (p, window-local chunk) | 128-col iota base]
    ib = int(tb[-1])
    cw = ib + P
    consts_arr = np.full((NCORES, P, cw), -1.0, np.float16)
    consts_arr[:, :, ib : ib + P] = np.arange(P, dtype=np.float16)
    consts_arr[core, p, tb[pos] + j] = (sidx - win * P).astype(np.float16)

    a_pad = np.zeros((N_PAD, D), np.float32)
    a_pad[:N] = A

    # Embed each window's 128 A rows right after its B rows.
    ce = np.repeat(np.arange(NCORES), WPC * P)
    pe_ = np.tile(np.repeat(np.arange(WPC), P), NCORES)
    v3 = np.tile(np.arange(P), NCORES * WPC)
    w3 = wid[ce, pe_]
    s3 = S[pe_] + pos_counts[ce, pe_] + v3
    a_rows = a_pad[w3 * P + v3].astype(np.float16)
    b_all[ce, s3 % P, s3 // P] = a_rows
    consts_arr[ce, s3 % P, tb[pe_] + s3 // P - c0[pe_]] = v3.astype(np.float16)

    b_flat = b_all.reshape(NCORES, P, tot_chunks * P)
    in_maps = [
        {"b_pad": b_flat[c], "consts": consts_arr[c]} for c in range(NCORES)
    ]
    return tuple(int(n) for n in need), wid, in_maps


def assemble_out(results, wid):
    """results[c]["out"] is (v, pos*128+d); route each position's window back
    to its window id's rows."""
    full = np.empty((N_PAD, D), np.float32)
    rows = full.reshape(W_PAD, P, D)
    for c in range(NCORES):
        o = np.asarray(results[c]["out"], dtype=np.float32)
        o = o.reshape(P, WPC, D).transpose(1, 0, 2)   # (pos, v, d)
        rows[wid[c]] = o
    return full[:N]


def kernel(index, A, B):
    from concourse.bass_utils import run_bass_kernel_spmd

    key, wid, in_maps = shard_inputs(index, A, B)
    if key not in _BUILT:
        _BUILT[key] = build_bass(key)
    nc = _BUILT[key]

    res = run_bass_kernel_spmd(nc, in_maps, list(range(NCORES)))
    global _LAST_RES
    _LAST_RES = res
    full = assemble_out(res.results, wid)
    return np.ascontiguousarray(full.astype(np.float32))
